# revision 1
# baseline (speedup 1.0000x reference)
"""Trainium2 Bass kernel for the fused sparse-attention block.

Computes (8-core SPMD, head-parallel + final row-shard re-layout):
    qkv = x @ W_qkv; q,k = rope(rmsnorm(q|k)); causal attention;
    out = (attn_out * sigmoid(x @ W_gate + b_gate)) @ W_out

Per core c (heads 2c, 2c+1 for both batches):
  Phase 1: PE-transpose x tiles -> xT; fused qkv+gate projection (f32r
           matmuls, feature-major output); RMSNorm (ones-matmul partition
           reduce) + RoPE; sigmoid gate; park qT/kT/gateT in DRAM scratch,
           keep v (transposed to natural) in SBUF.
  Phase 2: per (b,h): scoresT = kT.T-free QK matmul (no max subtraction --
           logits bounded), exp on ACT, causal mask via affine_select,
           PV + ones-denominator accumulation, normalize, gate multiply,
           write AllToAll bounce buffer.
  Phase 3: one AllToAll (head-shard -> row-shard), then row-sharded output
           projection with full W_out -> natural [512, 2048] shard.
"""
import sys
if '/opt/trn_rl_repo' not in sys.path:
    sys.path.insert(0, '/opt/trn_rl_repo')

import numpy as np


def _install_ntff_hook_shim():
    """Provide antenv.axon_hooks if the image lacks it (needed only when a
    caller requests NTFF tracing through run_bass_kernel_spmd)."""
    import types
    if 'antenv.axon_hooks' in sys.modules:
        return
    try:
        import antenv
    except ImportError:
        return
    if hasattr(antenv, 'axon_hooks'):
        return
    mod = types.ModuleType('antenv.axon_hooks')
    _state = {}

    def set_axon_ntff_profile_hook(h):
        _state['hook'] = h

    def get_axon_ntff_profile_hook():
        if 'hook' not in _state:
            try:
                from trn_agent_boot.trn_boot import _ntff_profile_via_ctypes
                _state['hook'] = _ntff_profile_via_ctypes('/opt/axon/libaxon_pjrt.so')
            except Exception:
                _state['hook'] = None
        return _state['hook']

    mod.set_axon_ntff_profile_hook = set_axon_ntff_profile_hook
    mod.get_axon_ntff_profile_hook = get_axon_ntff_profile_hook
    sys.modules['antenv.axon_hooks'] = mod
    antenv.axon_hooks = mod


_install_ntff_hook_shim()

B, T, D = 2, 2048, 2048
H = 16
d = 128
N_CORES = 8
HPC = H // N_CORES          # heads per core = 2
ROWS = B * T                # 4096
RC = 512                    # rows per phase-1 chunk
NRC = ROWS // RC            # 8 row chunks
KC = D // 128               # 16 contraction chunks
QKV_CT = 6                  # coltiles: q0 q1 k0 k1 v0 v1
GATE_CT = 2                 # g0 g1
NCT = QKV_CT + GATE_CT      # 8
QCH = 512                   # attention q chunk
EPS = 1e-6
ROPE_BASE = 10000.0
SCALE = 1.0 / np.sqrt(d)

_cache = {}


def _build():
    import concourse.bacc as bacc
    import concourse.mybir as mybir
    from concourse.tile import TileContext

    f32 = mybir.dt.float32
    f32r = mybir.dt.float32r
    bf16 = mybir.dt.bfloat16
    bf16 = mybir.dt.bfloat16
    AF = mybir.ActivationFunctionType

    nc = bacc.Bacc("TRN2", target_bir_lowering=False, debug=False,
                   num_devices=N_CORES)

    x_in = nc.dram_tensor("x", [ROWS, D], f32r, kind="ExternalInput").ap()
    w_in = nc.dram_tensor("w_qkvg", [D, NCT * 128], bf16, kind="ExternalInput").ap()
    wout_in = nc.dram_tensor("w_out", [D, D], bf16, kind="ExternalInput").ap()
    bg_in = nc.dram_tensor("b_gate", [128, HPC], f32, kind="ExternalInput").ap()
    cos_in = nc.dram_tensor("costab", [128, T], f32, kind="ExternalInput").ap()
    sin_in = nc.dram_tensor("sintab", [128, T], f32, kind="ExternalInput").ap()
    ident_in = nc.dram_tensor("ident", [128, 128], f32r, kind="ExternalInput").ap()
    mask_in = nc.dram_tensor("dmask", [128, 4 * QCH], f32, kind="ExternalInput").ap()
    out_ext = nc.dram_tensor("out", [RC, D], f32, kind="ExternalOutput").ap()

    def r_(ap):
        return ap.bitcast(f32r)

    with TileContext(nc) as tc:
        with tc.tile_pool(name="persist", bufs=1) as persist, \
             tc.tile_pool(name="p2k", bufs=1) as p2k, \
             tc.tile_pool(name="dram", bufs=1, space="DRAM") as dram:
            qT_park = [[dram.tile([128, T], f32r, name=f"qTp{h}_{b}")
                        for b in range(B)] for h in range(HPC)]
            kT_park = [[dram.tile([128, T], f32r, name=f"kTp{h}_{b}")
                        for b in range(B)] for h in range(HPC)]
            gT_park = [[dram.tile([128, T], f32, name=f"gTp{h}_{b}")
                        for b in range(B)] for h in range(HPC)]
            a2a_in = [dram.tile([N_CORES * 128, RC], bf16, name=f"a2a_in{h}")
                      for h in range(HPC)]
            a2a_out = [dram.tile([N_CORES * 128, RC], bf16, name=f"a2a_out{h}")
                      for h in range(HPC)]

            ident = persist.tile([128, 128], f32r, tag="ident")
            ones_col = persist.tile([128, 1], f32, tag="ones_col")
            ones_row = persist.tile([1, 128], f32, tag="ones_row")
            bg_sb = persist.tile([128, HPC], f32, tag="bg")
            # v natural, per head: v_sb[h][p, rt*128 + dd] = v[rt*128+p, dd]
            v_sb = [persist.tile([128, ROWS], f32r, tag=f"v{h}", name=f"v_sb{h}")
                     for h in range(HPC)]
            nc.sync.dma_start(out=ident[:], in_=ident_in[:])
            nc.sync.dma_start(out=bg_sb[:], in_=bg_in[:])
            eps_sb = persist.tile([128, 1], f32, tag="eps")
            mask_sb = persist.tile([128, 4 * QCH], f32, tag="mask")
            cc_sb = persist.tile([128, T], f32, tag="cc")
            ss_sb = persist.tile([128, T], f32, tag="ss")
            nc.vector.memset(ones_col[:], 1.0)
            nc.vector.memset(ones_row[:], 1.0)
            nc.vector.memset(eps_sb[:], EPS)
            nc.sync.dma_start(out=mask_sb[:], in_=mask_in[:])
            nc.sync.dma_start(out=cc_sb[:], in_=cos_in[:])
            nc.sync.dma_start(out=ss_sb[:], in_=sin_in[:])

            # ---------------- Phase 1 (p2 pools hoisted for virgin space) ----------------
            with tc.tile_pool(name="wq", bufs=1) as wq, \
                 tc.tile_pool(name="p1", bufs=2) as p1, \
                 tc.tile_pool(name="p1xt", bufs=3) as p1xt, \
                 tc.tile_pool(name="pp_t", bufs=2, space="PSUM") as pp_t, \
                 tc.tile_pool(name="pp_pj", bufs=4, space="PSUM") as pp_pj, \
                 tc.tile_pool(name="pp_st", bufs=1, space="PSUM") as pp_st, \
                 tc.tile_pool(name="pp_bc", bufs=1, space="PSUM") as pp_bc:
                # weights resident: col c of tile (k, ct) at w_sb[:, k*1024 + ct*128 + c]
                w_sb = wq.tile([128, KC * NCT * 128], bf16, tag="w")
                for k in range(KC):
                    nc.sync.dma_start(out=w_sb[:, k * 1024:(k + 1) * 1024],
                                      in_=w_in[k * 128:(k + 1) * 128, :])

                kT_all = {}
                for h in range(HPC):
                    for b in range(B):
                        kt_t = p2k.tile([128, T], f32r, tag=f"kT{h}{b}",
                                        name=f"kT_bh{h}{b}")
                        kT_all[(h, b)] = kt_t
                for rc in range(NRC):
                    b = rc // 4
                    t0 = (rc % 4) * RC

                    # transpose x chunk on PE: xT[k][p, r] = x[rc*512+r, k*128+p]
                    xth = [p1xt.tile([128, 8 * RC], bf16, tag="xt", name=f"xt_{rc}_{i}")
                           for i in range(2)]
                    for j in range(4):          # row tiles within chunk
                        xs = p1.tile([128, D], f32r, tag="xs")
                        nc.sync.dma_start(
                            out=xs[:], in_=x_in[rc * RC + j * 128: rc * RC + (j + 1) * 128, :])
                        for kq in range(4):     # groups of 4 dim-tiles
                            tp = pp_t.tile([128, 512], f32, tag="tp")
                            for u in range(4):
                                k = kq * 4 + u
                                nc.tensor.transpose(
                                    r_(tp[:, u * 128:(u + 1) * 128]),
                                    xs[:, k * 128:(k + 1) * 128], ident[:])
                            xt_view = xth[kq // 2][:].rearrange("p (k r) -> p k r", k=8)[
                                :, (kq % 2) * 4:(kq % 2) * 4 + 4, j * 128:(j + 1) * 128]
                            nc.vector.tensor_copy(
                                xt_view, tp[:].rearrange("p (u r) -> p u r", u=4))

                    # fused projection over 8 coltiles
                    for ct in range(NCT):
                        ps = pp_pj.tile([128, RC], f32, tag="pj")
                        for k in range(KC):
                            nc.tensor.matmul(
                                ps[:], w_sb[:, k * 1024 + ct * 128: k * 1024 + (ct + 1) * 128],
                                xth[k // 8][:, (k % 8) * RC:(k % 8 + 1) * RC],
                                start=(k == 0), stop=(k == KC - 1))
                        h = ct % 2
                        if ct < 4:
                            # q or k head: rmsnorm + rope
                            isq = ct < 2
                            sq = p1.tile([128, RC], f32r, tag="sq")
                            nc.scalar.activation(sq[:], ps[:], AF.Square)
                            ssq = pp_st.tile([1, RC], f32, tag="ssq")
                            nc.tensor.matmul(ssq[:], r_(ones_col[:]), sq[:],
                                             start=True, stop=True)
                            rstd = p1.tile([1, RC], f32r, tag="rstd")
                            nc.scalar.activation(rstd[:], ssq[:],
                                                 AF.Abs_reciprocal_sqrt,
                                                 scale=1.0 / 128.0,
                                                 bias=eps_sb[0:1, :])
                            bc = pp_bc.tile([128, RC], f32, tag="bc")
                            nc.tensor.matmul(bc[:], r_(ones_row[:]), rstd[:],
                                             start=True, stop=True)
                            bc_sb = p1.tile([128, RC], f32, tag="bc_sb")
                            nc.vector.tensor_copy(bc_sb[:], bc[:])
                            # normalize, then rope: fin = qn*cc + swap(qn)*ss
                            # (cc = [cos;cos], ss = [-sin;sin] host tables)
                            qn = p1.tile([128, RC], f32, tag="qn")
                            nc.vector.tensor_mul(qn[:], ps[:], bc_sb[:])
                            sw = p1.tile([128, RC], f32, tag="sw")
                            nc.sync.dma_start(out=sw[0:64, :], in_=qn[64:128, :])
                            nc.sync.dma_start(out=sw[64:128, :], in_=qn[0:64, :])
                            nc.vector.tensor_mul(qn[:], qn[:], cc_sb[:, t0:t0 + RC])
                            nc.vector.tensor_mul(sw[:], sw[:], ss_sb[:, t0:t0 + RC])
                            fin = p1.tile([128, RC], f32r, tag="fin")
                            nc.vector.tensor_add(fin[:], qn[:], sw[:])
                            park = qT_park if isq else kT_park
                            nc.sync.dma_start(
                                out=park[h][b][:, t0:t0 + RC], in_=fin[:])
                        elif ct < 6:
                            # v: psum -> sbuf, transpose to natural, evict
                            sv = p1.tile([128, RC], f32r, tag="sv")
                            nc.vector.tensor_copy(sv[:], ps[:])
                            tp = pp_t.tile([128, 512], f32, tag="tp")
                            for u in range(4):
                                nc.tensor.transpose(
                                    r_(tp[:, u * 128:(u + 1) * 128]),
                                    sv[:, u * 128:(u + 1) * 128], ident[:])
                            nc.vector.tensor_copy(
                                v_sb[h][:, rc * 4 * 128: (rc * 4 + 4) * 128], tp[:])
                        else:
                            # gate: sigmoid(ps + b)
                            gt = p1.tile([128, RC], f32, tag="gt")
                            nc.scalar.activation(gt[:], ps[:], AF.Sigmoid,
                                                 bias=bg_sb[:, h:h + 1])
                            nc.sync.dma_start(
                                out=gT_park[h][b][:, t0:t0 + RC], in_=gt[:])

                    if rc == 3 or rc == NRC - 1:
                        for hh in range(HPC):
                            nc.sync.dma_start(out=kT_all[(hh, b)][:],
                                              in_=kT_park[hh][b][:])

            # ---------------- Phase 2 + W_out prefetch ----------------
            with tc.tile_pool(name="wout", bufs=4) as woutp:
                wout_tiles = {}

                with tc.tile_pool(name="p2", bufs=2) as p2, \
                     tc.tile_pool(name="p2e", bufs=3) as p2e, \
                     tc.tile_pool(name="pp_s", bufs=3, space="PSUM") as pp_s, \
                     tc.tile_pool(name="pp_o", bufs=2, space="PSUM") as pp_o, \
                     tc.tile_pool(name="pp_d", bufs=2, space="PSUM") as pp_d, \
                     tc.tile_pool(name="pp_b", bufs=1, space="PSUM") as pp_b:
                    for h in range(HPC):
                        for b in range(B):
                            kT_bh = kT_all[(h, b)]
                            for qc in range(T // QCH):
                                col0 = qc * QCH
                                qTc = p2.tile([128, QCH], f32r, tag="qTc")
                                nc.sync.dma_start(
                                    out=qTc[:],
                                    in_=qT_park[h][b][:, col0:col0 + QCH])
                                o_ps = pp_o.tile([128, QCH], f32, tag="o")
                                den = pp_d.tile([1, QCH], f32, tag="den")
                                nkt = qc * 4 + 4
                                for kt in range(nkt):
                                    sc = pp_s.tile([128, QCH], f32, tag="sc")
                                    nc.tensor.matmul(
                                        sc[:], kT_bh[:, kt * 128:(kt + 1) * 128],
                                        qTc[:], start=True, stop=True)
                                    if kt >= qc * 4:
                                        m = kt - qc * 4
                                        nc.vector.tensor_add(
                                            sc[:], sc[:],
                                            mask_sb[:, m * QCH:(m + 1) * QCH])
                                    ex = p2e.tile([128, QCH], f32r, tag="ex")
                                    nc.scalar.activation(ex[:], sc[:], AF.Exp, scale=SCALE)
                                    nc.tensor.matmul(
                                        o_ps[:], v_sb[h][:, (b * 16 + kt) * 128:(b * 16 + kt + 1) * 128],
                                        ex[:], start=(kt == 0), stop=(kt == nkt - 1))
                                    nc.tensor.matmul(
                                        den[:], r_(ones_col[:]), ex[:],
                                        start=(kt == 0), stop=(kt == nkt - 1))
                                recip = p2.tile([1, QCH], f32r, tag="recip")
                                with nc.allow_low_precision(reason="fp32 recip"):
                                    nc.vector.reciprocal(recip[:], den[:])
                                bc2 = pp_b.tile([128, QCH], f32, tag="b2")
                                nc.tensor.matmul(bc2[:], r_(ones_row[:]), recip[:],
                                                 start=True, stop=True)
                                bc2_sb = p2.tile([128, QCH], f32, tag="b2sb")
                                nc.vector.tensor_copy(bc2_sb[:], bc2[:])
                                nm = p2.tile([128, QCH], f32, tag="nm")
                                nc.vector.tensor_mul(nm[:], o_ps[:], bc2_sb[:])
                                gT_sb = p2.tile([128, QCH], f32, tag="gT")
                                nc.sync.dma_start(
                                    out=gT_sb[:],
                                    in_=gT_park[h][b][:, col0:col0 + QCH])
                                on_sb = p2.tile([128, QCH], bf16, tag="onsb")
                                nc.vector.tensor_mul(on_sb[:], nm[:], gT_sb[:])
                                shard = b * 4 + qc
                                nc.sync.dma_start(
                                    out=a2a_in[h][shard * 128:(shard + 1) * 128, :],
                                    in_=on_sb[:])
                        nc.gpsimd.collective_compute(
                            "AllToAll", mybir.AluOpType.bypass,
                            replica_groups=[list(range(N_CORES))],
                            ins=[a2a_in[h].opt()], outs=[a2a_out[h].opt()])

                for oc in range(2):
                    wt = woutp.tile([128, KC * 512], bf16, tag="wo")
                    for k in range(KC):
                        nc.sync.dma_start(
                            out=wt[:, k * 512:(k + 1) * 512],
                            in_=wout_in[k * 128:(k + 1) * 128, oc * 512:(oc + 1) * 512])
                    wout_tiles[oc] = wt

                with tc.tile_pool(name="p3", bufs=1) as p3, \
                     tc.tile_pool(name="p3e", bufs=3) as p3e, \
                     tc.tile_pool(name="pp_3", bufs=8, space="PSUM") as pp_3:
                    gat = p3.tile([128, KC * RC], bf16, tag="gat")
                    for k in range(0, KC, 2):          # even: head-0 dims (A2A#1)
                        nc.sync.dma_start(
                            out=gat[:, k * RC:(k + 1) * RC],
                            in_=a2a_out[0][(k // 2) * 128:(k // 2 + 1) * 128, :])
                    for k in range(1, KC, 2):          # odd: head-1 dims (A2A#2)
                        nc.sync.dma_start(
                            out=gat[:, k * RC:(k + 1) * RC],
                            in_=a2a_out[1][(k // 2) * 128:(k // 2 + 1) * 128, :])
                    for oc in range(2, 4):
                        wt = woutp.tile([128, KC * 512], bf16, tag="wo")
                        for k in range(KC):
                            nc.sync.dma_start(
                                out=wt[:, k * 512:(k + 1) * 512],
                                in_=wout_in[k * 128:(k + 1) * 128, oc * 512:(oc + 1) * 512])
                        wout_tiles[oc] = wt
                    # pass A: even k (head-0 dims, ready after A2A#1) -> SBUF partials
                    partials = {}
                    for oc in range(4):
                        for rt in range(4):
                            ps = pp_3.tile([128, 512], f32, tag="o3",
                                           name=f"psA_{oc}_{rt}")
                            for k in range(0, KC, 2):
                                nc.tensor.matmul(
                                    ps[:], gat[:, k * RC + rt * 128: k * RC + (rt + 1) * 128],
                                    wout_tiles[oc][:, k * 512:(k + 1) * 512],
                                    start=(k == 0), stop=(k == KC - 2))
                            pa = p3.tile([128, 512], f32, tag=f"pa{oc}{rt}",
                                         name=f"pa_{oc}_{rt}")
                            nc.vector.tensor_copy(pa[:], ps[:])
                            partials[(oc, rt)] = pa
                    # pass B: odd k (head-1 dims, after A2A#2), add partial at evict
                    for oc in range(4):
                        for rt in range(4):
                            ps = pp_3.tile([128, 512], f32, tag="o3",
                                           name=f"psB_{oc}_{rt}")
                            for k in range(1, KC, 2):
                                nc.tensor.matmul(
                                    ps[:], gat[:, k * RC + rt * 128: k * RC + (rt + 1) * 128],
                                    wout_tiles[oc][:, k * 512:(k + 1) * 512],
                                    start=(k == 1), stop=(k == KC - 1))
                            ev = p3e.tile([128, 512], f32, tag="ev")
                            nc.vector.tensor_add(ev[:], ps[:], partials[(oc, rt)][:])
                            nc.sync.dma_start(
                                out=out_ext[rt * 128:(rt + 1) * 128, oc * 512:(oc + 1) * 512],
                                in_=ev[:])

    nc.compile()
    return nc


def _tables():
    inv = 1.0 / (ROPE_BASE ** (np.arange(0, d, 2, dtype=np.float64) / d))
    pos = np.arange(T, dtype=np.float64)
    ang = pos[None, :] * inv[:, None]          # [64, T]
    cos = np.cos(ang).astype(np.float32)
    sin = np.sin(ang).astype(np.float32)
    cc = np.concatenate([cos, cos], axis=0)    # [128, T]
    ss = np.concatenate([-sin, sin], axis=0)   # [128, T]
    return cc, ss


def kernel(x, W_qkv, W_out, W_gate, b_gate, mask):
    from concourse.bass_utils import run_bass_kernel_spmd

    if 'nc' not in _cache:
        _cache['nc'] = _build()
    nc = _cache['nc']

    x = np.ascontiguousarray(np.asarray(x, dtype=np.float32).reshape(ROWS, D))
    W_qkv = np.asarray(W_qkv, dtype=np.float32)
    import ml_dtypes
    W_out = np.ascontiguousarray(np.asarray(W_out, dtype=np.float32)).astype(ml_dtypes.bfloat16)
    W_gate = np.asarray(W_gate, dtype=np.float32)
    b_gate = np.asarray(b_gate, dtype=np.float32)
    cos, sin = _tables()
    ident = np.eye(128, dtype=np.float32)
    f = np.arange(QCH)[None, :]
    p = np.arange(128)[:, None]
    dmask = np.concatenate(
        [np.where(f >= p + 128 * m, 0.0, -1e9) for m in range(4)],
        axis=1).astype(np.float32)

    import ml_dtypes
    in_maps = []
    for c in range(N_CORES):
        h0 = HPC * c
        cols = []
        for kind in range(3):                     # q, k, v columns for this core's heads
            for h in range(h0, h0 + HPC):
                cols.append(W_qkv[:, kind * D + h * d:(kind * D + (h + 1) * d)])
        for h in range(h0, h0 + HPC):             # gate columns
            cols.append(W_gate[:, h * d:(h + 1) * d])
        w_qkvg = np.ascontiguousarray(
            np.concatenate(cols, axis=1)).astype(ml_dtypes.bfloat16)
        bg = np.ascontiguousarray(
            b_gate[h0 * d:(h0 + HPC) * d].reshape(HPC, 128).T)
        in_maps.append({
            "x": x, "w_qkvg": w_qkvg, "w_out": W_out, "b_gate": bg,
            "costab": cos, "sintab": sin, "ident": ident, "dmask": dmask,
        })

    res = run_bass_kernel_spmd(nc, in_maps, list(range(N_CORES)))
    _cache['last_results'] = res
    out = np.concatenate([res.results[c]["out"] for c in range(N_CORES)], axis=0)
    return out.reshape(B, T, D)

